# revision 48
# baseline (speedup 1.0000x reference)
"""Trainium2 Bass kernel for a dense transformer encoder layer.

Shapes (hardcoded): x (4, 2048, 1024), d_model=1024, n_head=16, head_dim=64,
d_ff=4096. 8 NeuronCores: core c handles batch c//2, query rows
(c%2)*1024:(c%2+1)*1024. K/V are computed per-core for the full batch
sequence (duplicated across the 2 cores of a batch pair) so there are no
collectives.

All activations are kept feature-major ("transposed": features on SBUF
partitions, tokens on the free axis) so every matmul contracts over the
partition dim. The Q/K path runs in float32r for precision; V, softmax
probabilities, Wo and the FFN run in bf16 with fp32 PSUM accumulation.
Softmax skips max-subtraction (scores are bounded ~|54|; exp fits fp32
easily); the denominator comes from a ones-column appended to V, and
normalization + bv are fused into the copy to the attention output.

SBUF is tight (192KB/partition), so long-lived tensors are chained through
shared tile-pool tags: tiles with the same tag reuse one slot sequentially.
"""

import os
import sys

import numpy as np

for _p in ("/opt/trn_rl_repo", "/root/.axon_site/_ro/trn_rl_repo"):
    if os.path.isdir(_p) and _p not in sys.path:
        sys.path.append(_p)

import ml_dtypes  # noqa: E402

import concourse.bass as bass  # noqa: E402
import concourse.tile as tile  # noqa: E402
from concourse import bacc, mybir  # noqa: E402
from concourse.masks import make_identity  # noqa: E402

F32 = mybir.dt.float32
F32R = mybir.dt.float32r
BF16 = mybir.dt.bfloat16
AF = mybir.ActivationFunctionType
ALU = mybir.AluOpType
AX = mybir.AxisListType

B, S, D = 4, 2048, 1024
H, HD = 16, 64
DFF = 4096
M = 1024  # query rows per core
P = 128
LN_EPS = 1e-5
SCALE_EPS = 1e-8
NCORES = 8

DK = D // P          # 8 feature tiles of d_model
SB = S // P          # 16 seq tiles of 128
QB = M // 512        # 2 query blocks of 512
VCOL = 65            # 64 v features + 1 ones column per head

# vecs packing: per-feature vectors, one column per 128-feature tile
C_BK, C_BQ, C_BV, C_BO, C_B2, C_G1, C_BE1, C_B1 = 0, 8, 16, 24, 32, 40, 48, 56
C_BVL = 88  # bv odd-half (partitions 64-127 of each tile) shifted to 0-63
VECS_COLS = 96


def _r(ap):  # f32 -> f32r view for matmul operands
    return ap.bitcast(F32R)


def _build_body(ctx, tc):
    nc = tc.nc
    ctx.enter_context(
        nc.allow_low_precision(reason="float32r carries full fp32 storage")
    )

    x_kvT = nc.dram_tensor("x_kvT", [D, S], F32R, kind="ExternalInput").ap()
    x_qT = nc.dram_tensor("x_qT", [D, M], F32R, kind="ExternalInput").ap()
    wqT_d = nc.dram_tensor("wqT", [D, D], F32R, kind="ExternalInput").ap()
    wkT_d = nc.dram_tensor("wkT", [D, D], F32R, kind="ExternalInput").ap()
    wvT_d = nc.dram_tensor("wvT", [D, D], BF16, kind="ExternalInput").ap()
    woT_d = nc.dram_tensor("woT", [D, D], BF16, kind="ExternalInput").ap()
    w1T_d = nc.dram_tensor("w1T", [D, DFF], BF16, kind="ExternalInput").ap()
    w2T_d = nc.dram_tensor("w2T", [DFF, D], BF16, kind="ExternalInput").ap()
    vecs_d = nc.dram_tensor("vecs", [P, VECS_COLS], F32, kind="ExternalInput").ap()
    g2b2_d = nc.dram_tensor("g2b2", [2, D], F32R, kind="ExternalInput").ap()
    out_d = nc.dram_tensor("out", [M, D], F32, kind="ExternalOutput").ap()

    xkv_t = x_kvT.rearrange("(a p) s -> a p s", p=P)   # (8, 128, 2048)
    xq_t = x_qT.rearrange("(a p) s -> a p s", p=P)     # (8, 128, 1024)
    wq_t = wqT_d.rearrange("(a p) f -> a p f", p=P)
    wk_t = wkT_d.rearrange("(a p) f -> a p f", p=P)
    wv_t = wvT_d.rearrange("(a p) f -> a p f", p=P)
    wo_t = woT_d.rearrange("(a p) f -> a p f", p=P)
    w1_t = w1T_d.rearrange("(a p) f -> a p f", p=P)    # (8, 128, 4096)
    w2_t = w2T_d.rearrange("(a p) f -> a p f", p=P)    # (32, 128, 1024)

    const = ctx.enter_context(tc.tile_pool(name="const", bufs=1))
    # lifetime-chained pools (same tag => same slot, reused sequentially)
    pA = ctx.enter_context(tc.tile_pool(name="pA", bufs=1))   # 32KB chain
    pB = ctx.enter_context(tc.tile_pool(name="pB", bufs=1))   # 16KB chain
    pC = ctx.enter_context(tc.tile_pool(name="pC", bufs=1))   # 32KB chain
    pD = ctx.enter_context(tc.tile_pool(name="pD", bufs=1))   # 16KB chain
    pV = ctx.enter_context(tc.tile_pool(name="pV", bufs=1))   # 32.5KB chain
    wproj = ctx.enter_context(tc.tile_pool(name="wproj", bufs=1))
    wff = ctx.enter_context(tc.tile_pool(name="wff", bufs=4))
    probs_pool = ctx.enter_context(tc.tile_pool(name="probs", bufs=8))
    xs = ctx.enter_context(tc.tile_pool(name="xs", bufs=4))
    scratch = ctx.enter_context(tc.tile_pool(name="scratch", bufs=2))
    rowp = ctx.enter_context(tc.tile_pool(name="rowp", bufs=1))
    scr1 = ctx.enter_context(tc.tile_pool(name="scr1", bufs=1))

    ps_acc = ctx.enter_context(tc.tile_pool(name="ps_acc", bufs=4, space="PSUM"))
    ps_sc = ctx.enter_context(tc.tile_pool(name="ps_sc", bufs=2, space="PSUM"))
    ps_misc = ctx.enter_context(tc.tile_pool(name="ps_misc", bufs=2, space="PSUM"))

    # ---- constants -------------------------------------------------------
    vecs = const.tile([P, VECS_COLS], F32)
    nc.sync.dma_start(vecs[:], vecs_d[:])
    identity = const.tile([P, P], F32)
    make_identity(nc, identity[:])
    headmask = const.tile([P, P], F32)
    nc.gpsimd.memset(headmask[:], 0.0)
    nc.gpsimd.memset(headmask[0:64, 0:64], 1.0)
    nc.gpsimd.memset(headmask[64:128, 64:128], 1.0)
    ones_col = const.tile([P, 1], F32)
    nc.gpsimd.memset(ones_col[:], 1.0)
    ones_row = const.tile([1, P], F32)
    nc.gpsimd.memset(ones_row[:], 1.0)
    inv_scale = const.tile([P, DK], F32)   # per-feature-tile 1/attn_scale
    bq_scaled = const.tile([P, DK], F32)
    eps_scale = const.tile([P, 1], F32)
    nc.gpsimd.memset(eps_scale[:], SCALE_EPS)
    eps_ln = const.tile([P, 1], F32)
    nc.gpsimd.memset(eps_ln[:], LN_EPS)
    g2_row = const.tile([1, D], F32R)
    nc.sync.dma_start(g2_row[:], g2b2_d[0:1, :])
    nc.vector.tensor_copy(g2_row[:], g2_row[:].bitcast(F32))
    be2_row = const.tile([1, D], F32R)
    nc.sync.dma_start(be2_row[:], g2b2_d[1:2, :])
    nc.vector.tensor_copy(be2_row[:], be2_row[:].bitcast(F32))
    # f32r twins of memset/gpsimd-built constants (fp32r matmul operands
    # must come from a rounding compute op)
    identity_r = const.tile([P, P], F32R)
    nc.vector.tensor_copy(identity_r[:], identity[:])
    ones_col_r = const.tile([P, 1], F32R)
    nc.vector.tensor_copy(ones_col_r[:], ones_col[:])
    ones_row_r = const.tile([1, P], F32R)
    nc.vector.tensor_copy(ones_row_r[:], ones_row[:])

    # persistent activations (chained slots)
    xkv_bf = pA.tile([P, DK, S], BF16, tag="chA", name="xkv_bf")     # 32KB
    wv_sb = pB.tile([P, DK, D], BF16, tag="chB", name="wv_sb")       # 16KB
    v_ones = pV.tile([P, SB, H * VCOL], BF16, tag="chV", name="v_ones")

    for dk in range(DK):
        nc.sync.dma_start(wv_sb[:, dk, :], wv_t[dk])
    for h in range(H):  # ones columns for the softmax denominator
        nc.gpsimd.memset(v_ones[:, :, h * VCOL + 64], 1.0)

    # =====================================================================
    # per head-group: K proj (+scale), Q proj, then attention
    # =====================================================================
    kT = {}
    qT = {}

    def k_proj(g):
        """K projection for head group g (feature tiles 4g..4g+3), f32r.

        Streams x_kvT from DRAM; on the g==0 pass also materializes the bf16
        copy of x_kvT used by the V projection. The attention 1/scale per
        head is computed here from the K psums (sum of squares -> per-head
        reduce via a block-diagonal mask matmul -> rsqrt), and folded into Q.
        """
        wk_sb = wproj.tile([P, DK, 512], F32R, tag="wproj", name=f"wk{g}")
        for dk in range(DK):
            nc.sync.dma_start(wk_sb[:, dk, :], wk_t[dk][:, g * 512:(g + 1) * 512])
            nc.vector.tensor_copy(wk_sb[:, dk, :],
                                  wk_sb[:, dk, :].bitcast(F32))
        kT[(g, 0)] = pC.tile([P, 2, S], F32R, tag="chC_a", name=f"kTa{g}")
        kT[(g, 1)] = pC.tile([P, 2, S], F32R, tag="chC_b", name=f"kTb{g}")
        sumsq = scr1.tile([P, 4, 4], F32, tag="sumsq")
        for sb4 in range(4):  # S blocks of 512
            psums = [ps_acc.tile([P, 512], F32, tag="acc", name=f"pk{sb4}_{i}")
                     for i in range(4)]
            for dk in range(DK):
                xkv_s = xs.tile([P, 512], F32R, tag="xs", name="xkv_s")
                nc.sync.dma_start(xkv_s[:], xkv_t[dk][:, sb4 * 512:(sb4 + 1) * 512])
                nc.vector.tensor_copy(xkv_s[:], xkv_s[:].bitcast(F32))
                if g == 0:
                    nc.vector.tensor_copy(
                        xkv_bf[:, dk, sb4 * 512:(sb4 + 1) * 512],
                        xkv_s[:].bitcast(F32),
                    )
                for m in range(4):
                    nc.tensor.matmul(
                        psums[m][:],
                        wk_sb[:, dk, m * 128:(m + 1) * 128],
                        xkv_s[:],
                        start=(dk == 0),
                        stop=(dk == DK - 1),
                    )
            for m in range(4):
                mg = g * 4 + m
                nc.vector.tensor_scalar_add(
                    kT[(g, m // 2)][:, m % 2, sb4 * 512:(sb4 + 1) * 512],
                    psums[m][:],
                    vecs[:, C_BK + mg:C_BK + mg + 1],
                )
                sq = scratch.tile([P, 512], F32, tag="sq")
                nc.scalar.activation(
                    sq[:],
                    kT[(g, m // 2)][:, m % 2, sb4 * 512:(sb4 + 1) * 512],
                    AF.Square,
                )
                nc.vector.tensor_reduce(
                    sumsq[:, m, sb4:sb4 + 1], sq[:], axis=AX.X, op=ALU.add
                )
        for m in range(4):
            mg = g * 4 + m
            ssq = scr1.tile([P, 1], F32, tag="ssq")
            nc.vector.tensor_reduce(ssq[:], sumsq[:, m, :], axis=AX.X, op=ALU.add)
            hs = ps_misc.tile([P, 1], F32, tag="misc", name=f"hs{m}")
            nc.tensor.matmul(hs[:], headmask[:], ssq[:], start=True,
                             stop=True)
            sc = scr1.tile([P, 1], F32, tag="ssq2")
            nc.scalar.activation(
                sc[:], hs[:], AF.Sqrt, bias=eps_scale[:, 0:1],
                scale=1.0 / (S * HD),
            )
            nc.vector.reciprocal(inv_scale[:, mg:mg + 1], sc[:])
            nc.vector.tensor_tensor(
                out=bq_scaled[:, mg:mg + 1],
                in0=vecs[:, C_BQ + mg:C_BQ + mg + 1],
                in1=inv_scale[:, mg:mg + 1],
                op=ALU.mult,
            )

    def q_proj(g):
        """Q projection for head group g; applies 1/scale (and bias) on copyout."""
        wq_sb = wproj.tile([P, DK, 512], F32R, tag="wproj", name=f"wq{g}")
        for dk in range(DK):
            nc.sync.dma_start(wq_sb[:, dk, :], wq_t[dk][:, g * 512:(g + 1) * 512])
            nc.vector.tensor_copy(wq_sb[:, dk, :],
                                  wq_sb[:, dk, :].bitcast(F32))
        qT[(g, 0)] = pD.tile([P, 2, M], F32R, tag="chD_a", name=f"qTa{g}")
        qT[(g, 1)] = pD.tile([P, 2, M], F32R, tag="chD_b", name=f"qTb{g}")
        for qb in range(QB):
            psums = [ps_acc.tile([P, 512], F32, tag="acc", name=f"pq{qb}_{i}")
                     for i in range(4)]
            for dk in range(DK):
                xq_s = xs.tile([P, 512], F32R, tag="xs", name="xq_s")
                nc.sync.dma_start(xq_s[:], xq_t[dk][:, qb * 512:(qb + 1) * 512])
                nc.vector.tensor_copy(xq_s[:], xq_s[:].bitcast(F32))
                for m in range(4):
                    nc.tensor.matmul(
                        psums[m][:],
                        wq_sb[:, dk, m * 128:(m + 1) * 128],
                        xq_s[:],
                        start=(dk == 0),
                        stop=(dk == DK - 1),
                    )
            for m in range(4):
                mg = g * 4 + m
                nc.vector.tensor_scalar(
                    out=qT[(g, m // 2)][:, m % 2, qb * 512:(qb + 1) * 512],
                    in0=psums[m][:],
                    scalar1=inv_scale[:, mg:mg + 1],
                    scalar2=bq_scaled[:, mg:mg + 1],
                    op0=ALU.mult,
                    op1=ALU.add,
                )

    def v_proj():
        """V projection (bf16), written into v_ones with stride-65 head blocks."""
        for sb in range(SB):
            for half in range(2):
                psv = ps_acc.tile([P, 512], F32, tag="acc", name="psv")
                for dk in range(DK):
                    nc.tensor.matmul(
                        psv[:],
                        xkv_bf[:, dk, sb * 128:(sb + 1) * 128],
                        wv_sb[:, dk, half * 512:(half + 1) * 512],
                        start=(dk == 0),
                        stop=(dk == DK - 1),
                    )
                vdst = v_ones[:, sb, :].rearrange(
                    "p (h c) -> p h c", c=VCOL
                )[:, half * 8:(half + 1) * 8, 0:64]
                nc.vector.tensor_copy(
                    vdst, psv[:].rearrange("p (h c) -> p h c", c=64)
                )

    def attention(g, attnout, qb):
        """Attention for head group g, query block qb. Scores via f32r (K=64;
        even/odd heads use different PE row groups and can overlap); exp on
        ACT straight out of PSUM into bf16 probs; attn@V against V+ones
        column gives the unnormalized output and the softmax denominator;
        normalization + bv fused into the copy to attnout."""
        for mt in range(4):  # feature tile in group = heads (2*mt, 2*mt+1)
            if True:
                pavs = {}
                for shalf in range(2):  # S halves of 8 tiles each
                    probs = {}
                    for sb8 in range(8):
                        sb = shalf * 8 + sb8
                        for ho in range(2):
                            pscore = ps_sc.tile([P, 512], F32, tag="sc",
                                                name=f"sc{sb}_{ho}")
                            nc.tensor.matmul(
                                pscore[:],
                                kT[(g, mt // 2)][ho * 64:ho * 64 + 64, mt % 2,
                                                 sb * 128:(sb + 1) * 128],
                                qT[(g, mt // 2)][ho * 64:ho * 64 + 64, mt % 2,
                                                 qb * 512:(qb + 1) * 512],
                                start=True,
                                stop=True,
                            )
                            pt = probs_pool.tile([P, 512], BF16, tag="probs",
                                                 name=f"pr{sb}_{ho}")
                            nc.scalar.activation(pt[:], pscore[:], AF.Exp)
                            probs[(sb8, ho)] = pt
                    if shalf == 0:
                        for ho in range(2):
                            pavs[ho] = ps_misc.tile([P, 512], F32, tag="misc",
                                                    name=f"pav{ho}")
                    for sb8 in range(8):
                        sb = shalf * 8 + sb8
                        for ho in range(2):
                            h = g * 8 + mt * 2 + ho
                            nc.tensor.matmul(
                                pavs[ho][0:VCOL, :],
                                v_ones[:, sb, h * VCOL:(h + 1) * VCOL],
                                probs[(sb8, ho)][:],
                                start=(sb == 0),
                                stop=(sb == SB - 1),
                            )
                for ho in range(2):
                    pav = pavs[ho]
                    rrow = rowp.tile([1, 512], F32R,
                                     tag=("mu" if ho == 0 else "var"),
                                     name=f"rrow{ho}")
                    recip = rrow[0:1, :]
                    nc.vector.reciprocal(recip, pav[64:65, :])
                    pbc = ps_acc.tile([P, 512], F32, tag="acc", name="pbc")
                    nc.tensor.matmul(
                        pbc[0:64, :], ones_row_r[:, 0:64], recip,
                        start=True, stop=True,
                    )
                    bc_sb = scratch.tile([64, 512], F32, tag="bcast_sb",
                                         name="bc_sb")
                    nc.vector.tensor_copy(bc_sb[:], pbc[0:64, :])
                    qsl = slice(qb * 512, (qb + 1) * 512)
                    if ho == 0:
                        dst = attnout[0:64, g * 4 + mt, qsl]
                        nc.vector.tensor_tensor(
                            out=dst, in0=pav[0:64, :], in1=bc_sb[:], op=ALU.mult
                        )
                        nc.vector.tensor_scalar_add(
                            dst, dst,
                            vecs[0:64, C_BV + g * 4 + mt:C_BV + g * 4 + mt + 1],
                        )
                    else:
                        # DVE lanes are partition-locked; normalize at 0-63
                        # then DMA-shift into partitions 64-127
                        ntmp = scr1.tile([64, 512], BF16, tag="ntmp",
                                            name="ntmp")
                        nc.vector.tensor_tensor(
                            out=ntmp[:], in0=pav[0:64, :], in1=bc_sb[:],
                            op=ALU.mult,
                        )
                        nc.vector.tensor_scalar_add(
                            ntmp[:], ntmp[:],
                            vecs[0:64, C_BVL + g * 4 + mt:C_BVL + g * 4 + mt + 1],
                        )
                        nc.sync.dma_start(
                            attnout[64:128, g * 4 + mt, qsl], ntmp[:]
                        )

    import os as _os
    _nphase = int(_os.environ.get("KERNEL_PHASES", "99"))

    k_proj(0)
    if _nphase < 2:
        return
    q_proj(0)
    if _nphase < 3:
        return
    v_proj()
    if _nphase < 4:
        return
    # attnout reuses the wv slot (chB); allocated after v_proj's last wv read
    attnout = pB.tile([P, DK, M], BF16, tag="chB", name="attnout")
    for qb in range(QB):
        attention(0, attnout, qb)
    if _nphase < 5:
        return
    k_proj(1)
    q_proj(1)
    for qb in range(QB):
        attention(1, attnout, qb)
    if _nphase < 6:
        return

    # =====================================================================
    # Wo projection + residual -> x1T; LayerNorm1 (in place on x1T); FFN;
    # transpose + LayerNorm2 + output DMA
    # =====================================================================
    wo_sb = wproj.tile([P, DK, D], BF16, tag="wproj", name="wo")
    for dk in range(DK):
        nc.sync.dma_start(wo_sb[:, dk, :], wo_t[dk])

    x1T = pA.tile([P, DK, M], F32R, tag="chA", name="x1T")
    xn1T_h = (pC.tile([P, DK // 2, M], F32, tag="chC_a", name="xn1Ta"),
              pC.tile([P, DK // 2, M], F32, tag="chC_b", name="xn1Tb"))
    xn1bf_h = (pD.tile([P, DK // 2, M], BF16, tag="chD_a", name="xn1bfa"),
               pD.tile([P, DK // 2, M], BF16, tag="chD_b", name="xn1bfb"))

    def xn1T(m):
        return xn1T_h[m // 4][:, m % 4, :]

    def xn1_bf(m):
        return xn1bf_h[m // 4][:, m % 4, :]

    for qb in range(QB):
        for m in range(DK):
            pp = ps_acc.tile([P, 512], F32, tag="acc", name=f"po{m}")
            for dk in range(DK):
                nc.tensor.matmul(
                    pp[:],
                    wo_sb[:, dk, m * 128:(m + 1) * 128],
                    attnout[:, dk, qb * 512:(qb + 1) * 512],
                    start=(dk == 0),
                    stop=(dk == DK - 1),
                )
            xres = xs.tile([P, 512], F32, tag="xs", name="xres")
            nc.sync.dma_start(
                xres[:], xq_t[m][:, qb * 512:(qb + 1) * 512].bitcast(F32)
            )
            dst = x1T[:, m, qb * 512:(qb + 1) * 512]
            nc.vector.tensor_scalar_add(dst, pp[:], vecs[:, C_BO + m:C_BO + m + 1])
            nc.vector.tensor_tensor(out=dst, in0=dst, in1=xres[:], op=ALU.add)

    def ln1_block(qb):
        """LayerNorm along the partition (feature) direction, in place on
        x1T: per-token stats via ones-matmuls, broadcast back via K=1
        matmuls; the bf16 copy for the FFN rhs is written alongside."""
        src_t = x1T
        psum_s = ps_misc.tile([1, 512], F32, tag="misc", name="lns")
        psum_q = ps_misc.tile([1, 512], F32, tag="misc", name="lnq")
        for m in range(DK):
            nc.tensor.matmul(
                psum_s[:], ones_col_r[:],
                src_t[:, m, qb * 512:(qb + 1) * 512],
                start=(m == 0), stop=(m == DK - 1),
            )
        for m in range(DK):
            sq = scratch.tile([P, 512], F32R, tag="sq")
            ssl = src_t[:, m, qb * 512:(qb + 1) * 512]
            nc.vector.tensor_tensor(out=sq[:], in0=ssl, in1=ssl, op=ALU.mult)
            nc.tensor.matmul(
                psum_q[:], ones_col_r[:], sq[:],
                start=(m == 0), stop=(m == DK - 1),
            )
        mu_t = rowp.tile([1, 512], F32R, tag="mu", name="mu_t")
        var_t = rowp.tile([1, 512], F32R, tag="var", name="var_t")
        rstd_t = rowp.tile([1, 512], F32R, tag="rstd", name="rstd_t")
        mu, var, rstd = mu_t[0:1, :], var_t[0:1, :], rstd_t[0:1, :]
        nc.vector.tensor_scalar_mul(mu, psum_s[:], 1.0 / D)
        # D*var = sum(x^2) - sum(x)*mu ; fold 1/D into the sqrt scale
        nc.vector.tensor_tensor(out=var, in0=psum_s[:], in1=mu, op=ALU.mult)
        nc.vector.tensor_tensor(out=var, in0=psum_q[:], in1=var,
                                op=ALU.subtract)
        nc.scalar.activation(var, var, AF.Sqrt, scale=1.0 / D,
                             bias=eps_ln[0:1, 0:1])
        nc.vector.reciprocal(rstd, var)
        pmu_ps = ps_sc.tile([P, 512], F32, tag="sc", name="pmu")
        nc.tensor.matmul(pmu_ps[:], ones_row_r[:], mu, start=True, stop=True)
        prstd_ps = ps_sc.tile([P, 512], F32, tag="sc", name="prstd")
        nc.tensor.matmul(prstd_ps[:], ones_row_r[:], rstd, start=True,
                         stop=True)
        pmu = scratch.tile([P, 512], F32, tag="bcast_sb", name="pmu_sb")
        nc.vector.tensor_copy(pmu[:], pmu_ps[:])
        prstd = scratch.tile([P, 512], F32, tag="bcast_sb", name="prstd_sb")
        nc.vector.tensor_copy(prstd[:], prstd_ps[:])
        qsl = slice(qb * 512, (qb + 1) * 512)
        for m in range(DK):
            t = scratch.tile([P, 512], F32, tag="lnt")
            nc.vector.tensor_tensor(out=t[:], in0=src_t[:, m, qsl],
                                    in1=pmu[:], op=ALU.subtract)
            nc.vector.tensor_tensor(out=t[:], in0=t[:], in1=prstd[:],
                                    op=ALU.mult)
            nc.vector.tensor_scalar(
                out=xn1T(m)[:, qsl], in0=t[:],
                scalar1=vecs[:, C_G1 + m:C_G1 + m + 1],
                scalar2=vecs[:, C_BE1 + m:C_BE1 + m + 1],
                op0=ALU.mult, op1=ALU.add,
            )
            nc.vector.tensor_copy(xn1_bf(m)[:, qsl], xn1T(m)[:, qsl])

    for qb in range(QB):
        ln1_block(qb)
    if _nphase < 7:
        return

    # ---- FFN (per q-half to halve y1 SBUF) ------------------------------
    x2T = pA.tile([P, DK, M], F32R, tag="chA", name="x2T")
    for qb in range(QB):
        y1 = pV.tile([P, DFF // P, 512], BF16, tag="chV", name=f"y1_{qb}")
        for mfg in range(8):  # groups of 4 dff tiles
            psums = [ps_acc.tile([P, 512], F32, tag="acc", name=f"pf{mfg}_{i}")
                     for i in range(4)]
            for dk in range(DK):
                w1s = wff.tile([P, 512], BF16, tag="w1s", name="w1s")
                nc.sync.dma_start(
                    w1s[:], w1_t[dk][:, mfg * 512:(mfg + 1) * 512]
                )
                for mf in range(4):
                    nc.tensor.matmul(
                        psums[mf][:],
                        w1s[:, mf * 128:(mf + 1) * 128],
                        xn1_bf(dk)[:, qb * 512:(qb + 1) * 512],
                        start=(dk == 0),
                        stop=(dk == DK - 1),
                    )
            for mf in range(4):
                mfg_g = mfg * 4 + mf
                nc.scalar.activation(
                    y1[:, mfg_g, :], psums[mf][:], AF.Relu,
                    bias=vecs[:, C_B1 + mfg_g:C_B1 + mfg_g + 1],
                )
        for mg in range(2):  # groups of 4 d_model tiles
            psums = [ps_acc.tile([P, 512], F32, tag="acc", name=f"pg{mg}_{i}")
                     for i in range(4)]
            for dk in range(DFF // P):
                w2s = wff.tile([P, 512], BF16, tag="w2s", name="w2s")
                nc.sync.dma_start(w2s[:], w2_t[dk][:, mg * 512:(mg + 1) * 512])
                for m2 in range(4):
                    nc.tensor.matmul(
                        psums[m2][:],
                        w2s[:, m2 * 128:(m2 + 1) * 128],
                        y1[:, dk, :],
                        start=(dk == 0),
                        stop=(dk == DFF // P - 1),
                    )
            for m2 in range(4):
                m = mg * 4 + m2
                dst = x2T[:, m, qb * 512:(qb + 1) * 512]
                nc.vector.tensor_scalar_add(
                    dst, psums[m2][:], vecs[:, C_B2 + m:C_B2 + m + 1]
                )
                nc.vector.tensor_tensor(
                    out=dst, in0=dst, in1=xn1T(m)[:, qb * 512:(qb + 1) * 512],
                    op=ALU.add,
                )

    if _nphase < 8:
        return
    # ---- transpose to natural layout, LayerNorm2, output ----------------
    gb_bcast = pB.tile([P, 2, D], F32, tag="chB", name="gb")
    for v, row in enumerate((g2_row, be2_row)):
        for half in range(2):
            pb = ps_sc.tile([P, 512], F32, tag="sc", name=f"gb{v}_{half}")
            nc.tensor.matmul(
                pb[:], ones_row_r[:],
                row[:, half * 512:(half + 1) * 512],
                start=True, stop=True,
            )
            nc.vector.tensor_copy(gb_bcast[:, v, half * 512:(half + 1) * 512],
                                  pb[:])

    for qt in range(M // P):
        natt = scratch.tile([P, D], F32, tag="nat", name=f"nat{qt}")
        nat = natt[:, :]
        for m in range(DK):
            ptr = ps_sc.tile([P, P], F32, tag="sc", name=f"ptr{m}")
            nc.tensor.transpose(
                _r(ptr[:]), x2T[:, m, qt * 128:(qt + 1) * 128], identity_r[:]
            )
            nc.vector.tensor_copy(nat[:, m * 128:(m + 1) * 128], ptr[:])
        ssum = scr1.tile([P, 1], F32, tag="nsum")
        nc.vector.tensor_reduce(ssum[:], nat[:], axis=AX.X, op=ALU.add)
        ssq = scr1.tile([P, 1], F32, tag="nssq")
        for half in range(2):
            sqh = scratch.tile([P, 512], F32, tag="sq", name=f"nsq{half}")
            nath = nat[:, half * 512:(half + 1) * 512]
            nc.vector.tensor_tensor(out=sqh[:], in0=nath, in1=nath,
                                    op=ALU.mult)
            if half == 0:
                nc.vector.tensor_reduce(ssq[:], sqh[:], axis=AX.X, op=ALU.add)
            else:
                s2 = scr1.tile([P, 1], F32, tag="nssq2")
                nc.vector.tensor_reduce(s2[:], sqh[:], axis=AX.X, op=ALU.add)
                nc.vector.tensor_tensor(out=ssq[:], in0=ssq[:], in1=s2[:],
                                        op=ALU.add)
        mu = scr1.tile([P, 1], F32, tag="nmu")
        nc.vector.tensor_scalar_mul(mu[:], ssum[:], 1.0 / D)
        msq = scr1.tile([P, 1], F32, tag="nmsq")
        nc.vector.tensor_scalar_mul(msq[:], ssq[:], 1.0 / D)
        var = scr1.tile([P, 1], F32, tag="nvar")
        nc.vector.tensor_tensor(out=var[:], in0=mu[:], in1=mu[:], op=ALU.mult)
        nc.vector.tensor_tensor(out=var[:], in0=msq[:], in1=var[:],
                                op=ALU.subtract)
        std = scr1.tile([P, 1], F32, tag="nstd")
        nc.scalar.activation(std[:], var[:], AF.Sqrt, bias=eps_ln[:, 0:1])
        rstd = scr1.tile([P, 1], F32, tag="nrstd")
        nc.vector.reciprocal(rstd[:], std[:])
        nc.vector.tensor_scalar(
            out=nat[:], in0=nat[:], scalar1=mu[:], scalar2=rstd[:],
            op0=ALU.subtract, op1=ALU.mult,
        )
        nc.vector.tensor_tensor(out=nat[:], in0=nat[:], in1=gb_bcast[:, 0, :],
                                op=ALU.mult)
        nc.vector.tensor_tensor(out=nat[:], in0=nat[:], in1=gb_bcast[:, 1, :],
                                op=ALU.add)
        nc.sync.dma_start(out_d[qt * 128:(qt + 1) * 128, :], nat)

_NC_CACHE = None


def build_nc():
    global _NC_CACHE
    if _NC_CACHE is not None:
        return _NC_CACHE
    from contextlib import ExitStack

    nc = bacc.Bacc("TRN2", target_bir_lowering=False, debug=False,
                   num_devices=NCORES)
    with tile.TileContext(nc) as tc:
        with ExitStack() as ctx:
            _build_body(ctx, tc)
    nc.compile()
    _NC_CACHE = nc
    return nc


def _pack_vec(v, ntiles):
    return np.ascontiguousarray(np.asarray(v, np.float32).reshape(ntiles, P).T)


def make_in_maps(inputs):
    f32 = lambda a: np.ascontiguousarray(np.asarray(a, np.float32))
    bf = lambda a: np.ascontiguousarray(
        np.asarray(a, np.float32).astype(ml_dtypes.bfloat16)
    )
    x = f32(inputs["x"])
    xT = np.ascontiguousarray(x.transpose(0, 2, 1))  # (B, D, S)

    vecs = np.zeros((P, VECS_COLS), np.float32)
    vecs[:, C_BK:C_BK + 8] = _pack_vec(inputs["bk"], 8)
    vecs[:, C_BQ:C_BQ + 8] = _pack_vec(inputs["bq"], 8)
    vecs[:, C_BV:C_BV + 8] = _pack_vec(inputs["bv"], 8)
    vecs[:, C_BO:C_BO + 8] = _pack_vec(inputs["bo"], 8)
    vecs[:, C_B2:C_B2 + 8] = _pack_vec(inputs["b2"], 8)
    vecs[:, C_G1:C_G1 + 8] = _pack_vec(inputs["g1"], 8)
    vecs[:, C_BE1:C_BE1 + 8] = _pack_vec(inputs["be1"], 8)
    vecs[:, C_B1:C_B1 + 32] = _pack_vec(inputs["b1"], 32)
    bv_t = _pack_vec(inputs["bv"], 8)  # (128, 8)
    vecs[0:64, C_BVL:C_BVL + 8] = bv_t[64:128, :]

    shared = {
        "wqT": f32(np.asarray(inputs["wq"], np.float32).T),
        "wkT": f32(np.asarray(inputs["wk"], np.float32).T),
        "wvT": bf(np.asarray(inputs["wv"], np.float32).T),
        "woT": bf(np.asarray(inputs["wo"], np.float32).T),
        "w1T": bf(np.asarray(inputs["w1"], np.float32).T),
        "w2T": bf(np.asarray(inputs["w2"], np.float32).T),
        "vecs": vecs,
        "g2b2": np.ascontiguousarray(
            np.stack([f32(inputs["g2"]), f32(inputs["be2"])])
        ),
    }
    in_maps = []
    for c in range(NCORES):
        b, half = c // 2, c % 2
        m = dict(shared)
        m["x_kvT"] = np.ascontiguousarray(xT[b])
        m["x_qT"] = np.ascontiguousarray(xT[b][:, half * M:(half + 1) * M])
        in_maps.append(m)
    return in_maps


def run(inputs, trace=False, **kw):
    from concourse.bass_utils import run_bass_kernel_spmd

    nc = build_nc()
    in_maps = make_in_maps(inputs)
    res = run_bass_kernel_spmd(nc, in_maps, core_ids=list(range(NCORES)),
                               trace=trace, **kw)
    out = np.empty((B, S, D), np.float32)
    for c in range(NCORES):
        b, half = c // 2, c % 2
        out[b, half * M:(half + 1) * M, :] = res.results[c]["out"]
    return out, res


def kernel(**inputs):
    out, _ = run(inputs)
    return out


# revision 49
# speedup vs baseline: 1.0075x; 1.0075x over previous
"""Trainium2 Bass kernel for a dense transformer encoder layer.

Shapes (hardcoded): x (4, 2048, 1024), d_model=1024, n_head=16, head_dim=64,
d_ff=4096. 8 NeuronCores: core c handles batch c//2, query rows
(c%2)*1024:(c%2+1)*1024. K/V are computed per-core for the full batch
sequence (duplicated across the 2 cores of a batch pair) so there are no
collectives.

All activations are kept feature-major ("transposed": features on SBUF
partitions, tokens on the free axis) so every matmul contracts over the
partition dim. The Q/K path runs in float32r for precision; V, softmax
probabilities, Wo and the FFN run in bf16 with fp32 PSUM accumulation.
Softmax skips max-subtraction (scores are bounded ~|54|; exp fits fp32
easily); the denominator comes from a ones-column appended to V, and
normalization + bv are fused into the copy to the attention output.

SBUF is tight (192KB/partition), so long-lived tensors are chained through
shared tile-pool tags: tiles with the same tag reuse one slot sequentially.
"""

import os
import sys

import numpy as np

for _p in ("/opt/trn_rl_repo", "/root/.axon_site/_ro/trn_rl_repo"):
    if os.path.isdir(_p) and _p not in sys.path:
        sys.path.append(_p)

import ml_dtypes  # noqa: E402

import concourse.bass as bass  # noqa: E402
import concourse.tile as tile  # noqa: E402
from concourse import bacc, mybir  # noqa: E402
from concourse.masks import make_identity  # noqa: E402

F32 = mybir.dt.float32
F32R = mybir.dt.float32r
BF16 = mybir.dt.bfloat16
AF = mybir.ActivationFunctionType
ALU = mybir.AluOpType
AX = mybir.AxisListType

B, S, D = 4, 2048, 1024
H, HD = 16, 64
DFF = 4096
M = 1024  # query rows per core
P = 128
LN_EPS = 1e-5
SCALE_EPS = 1e-8
NCORES = 8

DK = D // P          # 8 feature tiles of d_model
SB = S // P          # 16 seq tiles of 128
QB = M // 512        # 2 query blocks of 512
VCOL = 65            # 64 v features + 1 ones column per head

# vecs packing: per-feature vectors, one column per 128-feature tile
C_BK, C_BQ, C_BV, C_BO, C_B2, C_G1, C_BE1, C_B1 = 0, 8, 16, 24, 32, 40, 48, 56
C_BVL = 88  # bv odd-half (partitions 64-127 of each tile) shifted to 0-63
VECS_COLS = 96


def _r(ap):  # f32 -> f32r view for matmul operands
    return ap.bitcast(F32R)


def _build_body(ctx, tc):
    nc = tc.nc
    ctx.enter_context(
        nc.allow_low_precision(reason="float32r carries full fp32 storage")
    )

    x_kvT = nc.dram_tensor("x_kvT", [D, S], F32R, kind="ExternalInput").ap()
    x_qT = nc.dram_tensor("x_qT", [D, M], F32R, kind="ExternalInput").ap()
    wqT_d = nc.dram_tensor("wqT", [D, D], F32R, kind="ExternalInput").ap()
    wkT_d = nc.dram_tensor("wkT", [D, D], F32R, kind="ExternalInput").ap()
    wvT_d = nc.dram_tensor("wvT", [D, D], BF16, kind="ExternalInput").ap()
    woT_d = nc.dram_tensor("woT", [D, D], BF16, kind="ExternalInput").ap()
    w1T_d = nc.dram_tensor("w1T", [D, DFF], BF16, kind="ExternalInput").ap()
    w2T_d = nc.dram_tensor("w2T", [DFF, D], BF16, kind="ExternalInput").ap()
    vecs_d = nc.dram_tensor("vecs", [P, VECS_COLS], F32, kind="ExternalInput").ap()
    g2b2_d = nc.dram_tensor("g2b2", [2, D], F32R, kind="ExternalInput").ap()
    out_d = nc.dram_tensor("out", [M, D], F32, kind="ExternalOutput").ap()

    xkv_t = x_kvT.rearrange("(a p) s -> a p s", p=P)   # (8, 128, 2048)
    xq_t = x_qT.rearrange("(a p) s -> a p s", p=P)     # (8, 128, 1024)
    wq_t = wqT_d.rearrange("(a p) f -> a p f", p=P)
    wk_t = wkT_d.rearrange("(a p) f -> a p f", p=P)
    wv_t = wvT_d.rearrange("(a p) f -> a p f", p=P)
    wo_t = woT_d.rearrange("(a p) f -> a p f", p=P)
    w1_t = w1T_d.rearrange("(a p) f -> a p f", p=P)    # (8, 128, 4096)
    w2_t = w2T_d.rearrange("(a p) f -> a p f", p=P)    # (32, 128, 1024)

    const = ctx.enter_context(tc.tile_pool(name="const", bufs=1))
    # lifetime-chained pools (same tag => same slot, reused sequentially)
    pA = ctx.enter_context(tc.tile_pool(name="pA", bufs=1))   # 32KB chain
    pB = ctx.enter_context(tc.tile_pool(name="pB", bufs=1))   # 16KB chain
    pC = ctx.enter_context(tc.tile_pool(name="pC", bufs=1))   # 32KB chain
    pD = ctx.enter_context(tc.tile_pool(name="pD", bufs=1))   # 16KB chain
    pV = ctx.enter_context(tc.tile_pool(name="pV", bufs=1))   # 32.5KB chain
    wproj = ctx.enter_context(tc.tile_pool(name="wproj", bufs=1))
    wff = ctx.enter_context(tc.tile_pool(name="wff", bufs=4))
    probs_pool = ctx.enter_context(tc.tile_pool(name="probs", bufs=8))
    xs = ctx.enter_context(tc.tile_pool(name="xs", bufs=4))
    scratch = ctx.enter_context(tc.tile_pool(name="scratch", bufs=2))
    rowp = ctx.enter_context(tc.tile_pool(name="rowp", bufs=1))
    scr1 = ctx.enter_context(tc.tile_pool(name="scr1", bufs=1))

    ps_acc = ctx.enter_context(tc.tile_pool(name="ps_acc", bufs=4, space="PSUM"))
    ps_sc = ctx.enter_context(tc.tile_pool(name="ps_sc", bufs=2, space="PSUM"))
    ps_misc = ctx.enter_context(tc.tile_pool(name="ps_misc", bufs=2, space="PSUM"))

    # ---- constants -------------------------------------------------------
    vecs = const.tile([P, VECS_COLS], F32)
    nc.sync.dma_start(vecs[:], vecs_d[:])
    identity = const.tile([P, P], F32)
    make_identity(nc, identity[:])
    headmask = const.tile([P, P], F32)
    nc.gpsimd.memset(headmask[:], 0.0)
    nc.gpsimd.memset(headmask[0:64, 0:64], 1.0)
    nc.gpsimd.memset(headmask[64:128, 64:128], 1.0)
    ones_col = const.tile([P, 1], F32)
    nc.gpsimd.memset(ones_col[:], 1.0)
    ones_row = const.tile([1, P], F32)
    nc.gpsimd.memset(ones_row[:], 1.0)
    inv_scale = const.tile([P, DK], F32)   # per-feature-tile 1/attn_scale
    bq_scaled = const.tile([P, DK], F32)
    eps_scale = const.tile([P, 1], F32)
    nc.gpsimd.memset(eps_scale[:], SCALE_EPS)
    eps_ln = const.tile([P, 1], F32)
    nc.gpsimd.memset(eps_ln[:], LN_EPS)
    g2_row = const.tile([1, D], F32R)
    nc.sync.dma_start(g2_row[:], g2b2_d[0:1, :])
    nc.vector.tensor_copy(g2_row[:], g2_row[:].bitcast(F32))
    be2_row = const.tile([1, D], F32R)
    nc.sync.dma_start(be2_row[:], g2b2_d[1:2, :])
    nc.vector.tensor_copy(be2_row[:], be2_row[:].bitcast(F32))
    # f32r twins of memset/gpsimd-built constants (fp32r matmul operands
    # must come from a rounding compute op)
    identity_r = const.tile([P, P], F32R)
    nc.vector.tensor_copy(identity_r[:], identity[:])
    ones_col_r = const.tile([P, 1], F32R)
    nc.vector.tensor_copy(ones_col_r[:], ones_col[:])
    ones_row_r = const.tile([1, P], F32R)
    nc.vector.tensor_copy(ones_row_r[:], ones_row[:])

    # persistent activations (chained slots)
    xkv_bf = pA.tile([P, DK, S], BF16, tag="chA", name="xkv_bf")     # 32KB
    wv_sb = pB.tile([P, DK, D], BF16, tag="chB", name="wv_sb")       # 16KB
    v_ones = pV.tile([P, SB, H * VCOL], BF16, tag="chV", name="v_ones")

    for dk in range(DK):
        nc.sync.dma_start(wv_sb[:, dk, :], wv_t[dk])
    for h in range(H):  # ones columns for the softmax denominator
        nc.gpsimd.memset(v_ones[:, :, h * VCOL + 64], 1.0)

    # =====================================================================
    # per head-group: K proj (+scale), Q proj, then attention
    # =====================================================================
    kT = {}
    qT = {}

    def k_proj(g):
        """K projection for head group g (feature tiles 4g..4g+3), f32r.

        Streams x_kvT from DRAM; on the g==0 pass also materializes the bf16
        copy of x_kvT used by the V projection. The attention 1/scale per
        head is computed here from the K psums (sum of squares -> per-head
        reduce via a block-diagonal mask matmul -> rsqrt), and folded into Q.
        """
        wk_sb = wproj.tile([P, DK, 512], F32R, tag="wproj", name=f"wk{g}")
        for dk in range(DK):
            nc.sync.dma_start(wk_sb[:, dk, :], wk_t[dk][:, g * 512:(g + 1) * 512])
            nc.vector.tensor_copy(wk_sb[:, dk, :],
                                  wk_sb[:, dk, :].bitcast(F32))
        kT[(g, 0)] = pC.tile([P, 2, S], F32R, tag="chC_a", name=f"kTa{g}")
        kT[(g, 1)] = pC.tile([P, 2, S], F32R, tag="chC_b", name=f"kTb{g}")
        sumsq = scr1.tile([P, 4, 4], F32, tag="sumsq")
        for sb4 in range(4):  # S blocks of 512
            psums = [ps_acc.tile([P, 512], F32, tag="acc", name=f"pk{sb4}_{i}")
                     for i in range(4)]
            for dk in range(DK):
                xkv_s = xs.tile([P, 512], F32R, tag="xs", name="xkv_s")
                nc.sync.dma_start(xkv_s[:], xkv_t[dk][:, sb4 * 512:(sb4 + 1) * 512])
                nc.vector.tensor_copy(xkv_s[:], xkv_s[:].bitcast(F32))
                if g == 0:
                    nc.vector.tensor_copy(
                        xkv_bf[:, dk, sb4 * 512:(sb4 + 1) * 512],
                        xkv_s[:].bitcast(F32),
                    )
                for m in range(4):
                    nc.tensor.matmul(
                        psums[m][:],
                        wk_sb[:, dk, m * 128:(m + 1) * 128],
                        xkv_s[:],
                        start=(dk == 0),
                        stop=(dk == DK - 1),
                    )
            for m in range(4):
                mg = g * 4 + m
                nc.vector.tensor_scalar_add(
                    kT[(g, m // 2)][:, m % 2, sb4 * 512:(sb4 + 1) * 512],
                    psums[m][:],
                    vecs[:, C_BK + mg:C_BK + mg + 1],
                )
                sq = scratch.tile([P, 512], F32, tag="sq")
                nc.scalar.activation(
                    sq[:],
                    kT[(g, m // 2)][:, m % 2, sb4 * 512:(sb4 + 1) * 512],
                    AF.Square,
                )
                nc.vector.tensor_reduce(
                    sumsq[:, m, sb4:sb4 + 1], sq[:], axis=AX.X, op=ALU.add
                )
        for m in range(4):
            mg = g * 4 + m
            ssq = scr1.tile([P, 1], F32, tag="ssq")
            nc.vector.tensor_reduce(ssq[:], sumsq[:, m, :], axis=AX.X, op=ALU.add)
            hs = ps_misc.tile([P, 1], F32, tag="misc", name=f"hs{m}")
            nc.tensor.matmul(hs[:], headmask[:], ssq[:], start=True,
                             stop=True)
            sc = scr1.tile([P, 1], F32, tag="ssq2")
            nc.scalar.activation(
                sc[:], hs[:], AF.Sqrt, bias=eps_scale[:, 0:1],
                scale=1.0 / (S * HD),
            )
            nc.vector.reciprocal(inv_scale[:, mg:mg + 1], sc[:])
            nc.vector.tensor_tensor(
                out=bq_scaled[:, mg:mg + 1],
                in0=vecs[:, C_BQ + mg:C_BQ + mg + 1],
                in1=inv_scale[:, mg:mg + 1],
                op=ALU.mult,
            )

    def q_proj(g):
        """Q projection for head group g; applies 1/scale (and bias) on copyout."""
        wq_sb = wproj.tile([P, DK, 512], F32R, tag="wproj", name=f"wq{g}")
        for dk in range(DK):
            nc.sync.dma_start(wq_sb[:, dk, :], wq_t[dk][:, g * 512:(g + 1) * 512])
            nc.vector.tensor_copy(wq_sb[:, dk, :],
                                  wq_sb[:, dk, :].bitcast(F32))
        qT[(g, 0)] = pD.tile([P, 2, M], F32R, tag="chD_a", name=f"qTa{g}")
        qT[(g, 1)] = pD.tile([P, 2, M], F32R, tag="chD_b", name=f"qTb{g}")
        for qb in range(QB):
            psums = [ps_acc.tile([P, 512], F32, tag="acc", name=f"pq{qb}_{i}")
                     for i in range(4)]
            for dk in range(DK):
                xq_s = xs.tile([P, 512], F32R, tag="xs", name="xq_s")
                nc.sync.dma_start(xq_s[:], xq_t[dk][:, qb * 512:(qb + 1) * 512])
                nc.vector.tensor_copy(xq_s[:], xq_s[:].bitcast(F32))
                for m in range(4):
                    nc.tensor.matmul(
                        psums[m][:],
                        wq_sb[:, dk, m * 128:(m + 1) * 128],
                        xq_s[:],
                        start=(dk == 0),
                        stop=(dk == DK - 1),
                    )
            for m in range(4):
                mg = g * 4 + m
                nc.vector.tensor_scalar(
                    out=qT[(g, m // 2)][:, m % 2, qb * 512:(qb + 1) * 512],
                    in0=psums[m][:],
                    scalar1=inv_scale[:, mg:mg + 1],
                    scalar2=bq_scaled[:, mg:mg + 1],
                    op0=ALU.mult,
                    op1=ALU.add,
                )

    def v_proj():
        """V projection (bf16), written into v_ones with stride-65 head blocks."""
        for sb in range(SB):
            for half in range(2):
                psv = ps_acc.tile([P, 512], F32, tag="acc", name="psv")
                for dk in range(DK):
                    nc.tensor.matmul(
                        psv[:],
                        xkv_bf[:, dk, sb * 128:(sb + 1) * 128],
                        wv_sb[:, dk, half * 512:(half + 1) * 512],
                        start=(dk == 0),
                        stop=(dk == DK - 1),
                    )
                vdst = v_ones[:, sb, :].rearrange(
                    "p (h c) -> p h c", c=VCOL
                )[:, half * 8:(half + 1) * 8, 0:64]
                nc.vector.tensor_copy(
                    vdst, psv[:].rearrange("p (h c) -> p h c", c=64)
                )

    def attention(g, attnout, qb):
        """Attention for head group g, query block qb. Scores via f32r (K=64;
        even/odd heads use different PE row groups and can overlap); exp on
        ACT straight out of PSUM into bf16 probs; attn@V against V+ones
        column gives the unnormalized output and the softmax denominator;
        normalization + bv fused into the copy to attnout."""
        for mt in range(4):  # feature tile in group = heads (2*mt, 2*mt+1)
            if True:
                pavs = {}
                for shalf in range(2):  # S halves of 8 tiles each
                    probs = {}
                    for sb8 in range(8):
                        sb = shalf * 8 + sb8
                        for ho in range(2):
                            pscore = ps_sc.tile([P, 512], F32, tag="sc",
                                                name=f"sc{sb}_{ho}")
                            nc.tensor.matmul(
                                pscore[:],
                                kT[(g, mt // 2)][ho * 64:ho * 64 + 64, mt % 2,
                                                 sb * 128:(sb + 1) * 128],
                                qT[(g, mt // 2)][ho * 64:ho * 64 + 64, mt % 2,
                                                 qb * 512:(qb + 1) * 512],
                                start=True,
                                stop=True,
                            )
                            pt = probs_pool.tile([P, 512], BF16, tag="probs",
                                                 name=f"pr{sb}_{ho}")
                            nc.scalar.activation(pt[:], pscore[:], AF.Exp)
                            probs[(sb8, ho)] = pt
                    if shalf == 0:
                        for ho in range(2):
                            pavs[ho] = ps_misc.tile([P, 512], F32, tag="misc",
                                                    name=f"pav{ho}")
                    for sb8 in range(8):
                        sb = shalf * 8 + sb8
                        for ho in range(2):
                            h = g * 8 + mt * 2 + ho
                            nc.tensor.matmul(
                                pavs[ho][0:VCOL, :],
                                v_ones[:, sb, h * VCOL:(h + 1) * VCOL],
                                probs[(sb8, ho)][:],
                                start=(sb == 0),
                                stop=(sb == SB - 1),
                            )
                for ho in range(2):
                    pav = pavs[ho]
                    rrow = rowp.tile([1, 512], F32R,
                                     tag=("mu" if ho == 0 else "var"),
                                     name=f"rrow{ho}")
                    recip = rrow[0:1, :]
                    nc.vector.reciprocal(recip, pav[64:65, :])
                    pbc = ps_acc.tile([P, 512], F32, tag="acc", name="pbc")
                    nc.tensor.matmul(
                        pbc[0:64, :], ones_row_r[:, 0:64], recip,
                        start=True, stop=True,
                    )
                    bc_sb = scratch.tile([64, 512], F32, tag="bcast_sb",
                                         name="bc_sb")
                    nc.vector.tensor_copy(bc_sb[:], pbc[0:64, :])
                    qsl = slice(qb * 512, (qb + 1) * 512)
                    if ho == 0:
                        dst = attnout[0:64, g * 4 + mt, qsl]
                        nc.vector.tensor_tensor(
                            out=dst, in0=pav[0:64, :], in1=bc_sb[:], op=ALU.mult
                        )
                        nc.vector.tensor_scalar_add(
                            dst, dst,
                            vecs[0:64, C_BV + g * 4 + mt:C_BV + g * 4 + mt + 1],
                        )
                    else:
                        # DVE lanes are partition-locked; normalize at 0-63
                        # then DMA-shift into partitions 64-127
                        ntmp = scr1.tile([64, 512], BF16, tag="ntmp",
                                            name="ntmp")
                        nc.vector.tensor_tensor(
                            out=ntmp[:], in0=pav[0:64, :], in1=bc_sb[:],
                            op=ALU.mult,
                        )
                        nc.vector.tensor_scalar_add(
                            ntmp[:], ntmp[:],
                            vecs[0:64, C_BVL + g * 4 + mt:C_BVL + g * 4 + mt + 1],
                        )
                        nc.sync.dma_start(
                            attnout[64:128, g * 4 + mt, qsl], ntmp[:]
                        )

    import os as _os
    _nphase = int(_os.environ.get("KERNEL_PHASES", "99"))

    k_proj(0)
    if _nphase < 2:
        return
    v_proj()
    if _nphase < 3:
        return
    q_proj(0)
    if _nphase < 4:
        return
    # attnout reuses the wv slot (chB); allocated after v_proj's last wv read
    attnout = pB.tile([P, DK, M], BF16, tag="chB", name="attnout")
    for qb in range(QB):
        attention(0, attnout, qb)
    if _nphase < 5:
        return
    k_proj(1)
    q_proj(1)
    for qb in range(QB):
        attention(1, attnout, qb)
    if _nphase < 6:
        return

    # =====================================================================
    # Wo projection + residual -> x1T; LayerNorm1 (in place on x1T); FFN;
    # transpose + LayerNorm2 + output DMA
    # =====================================================================
    wo_sb = wproj.tile([P, DK, D], BF16, tag="wproj", name="wo")
    for dk in range(DK):
        nc.sync.dma_start(wo_sb[:, dk, :], wo_t[dk])

    x1T = pA.tile([P, DK, M], F32R, tag="chA", name="x1T")
    xn1T_h = (pC.tile([P, DK // 2, M], F32, tag="chC_a", name="xn1Ta"),
              pC.tile([P, DK // 2, M], F32, tag="chC_b", name="xn1Tb"))
    xn1bf_h = (pD.tile([P, DK // 2, M], BF16, tag="chD_a", name="xn1bfa"),
               pD.tile([P, DK // 2, M], BF16, tag="chD_b", name="xn1bfb"))

    def xn1T(m):
        return xn1T_h[m // 4][:, m % 4, :]

    def xn1_bf(m):
        return xn1bf_h[m // 4][:, m % 4, :]

    for qb in range(QB):
        for m in range(DK):
            pp = ps_acc.tile([P, 512], F32, tag="acc", name=f"po{m}")
            for dk in range(DK):
                nc.tensor.matmul(
                    pp[:],
                    wo_sb[:, dk, m * 128:(m + 1) * 128],
                    attnout[:, dk, qb * 512:(qb + 1) * 512],
                    start=(dk == 0),
                    stop=(dk == DK - 1),
                )
            xres = xs.tile([P, 512], F32, tag="xs", name="xres")
            nc.sync.dma_start(
                xres[:], xq_t[m][:, qb * 512:(qb + 1) * 512].bitcast(F32)
            )
            dst = x1T[:, m, qb * 512:(qb + 1) * 512]
            nc.vector.tensor_scalar_add(dst, pp[:], vecs[:, C_BO + m:C_BO + m + 1])
            nc.vector.tensor_tensor(out=dst, in0=dst, in1=xres[:], op=ALU.add)

    def ln1_block(qb):
        """LayerNorm along the partition (feature) direction, in place on
        x1T: per-token stats via ones-matmuls, broadcast back via K=1
        matmuls; the bf16 copy for the FFN rhs is written alongside."""
        src_t = x1T
        psum_s = ps_misc.tile([1, 512], F32, tag="misc", name="lns")
        psum_q = ps_misc.tile([1, 512], F32, tag="misc", name="lnq")
        for m in range(DK):
            nc.tensor.matmul(
                psum_s[:], ones_col_r[:],
                src_t[:, m, qb * 512:(qb + 1) * 512],
                start=(m == 0), stop=(m == DK - 1),
            )
        for m in range(DK):
            sq = scratch.tile([P, 512], F32R, tag="sq")
            ssl = src_t[:, m, qb * 512:(qb + 1) * 512]
            nc.vector.tensor_tensor(out=sq[:], in0=ssl, in1=ssl, op=ALU.mult)
            nc.tensor.matmul(
                psum_q[:], ones_col_r[:], sq[:],
                start=(m == 0), stop=(m == DK - 1),
            )
        mu_t = rowp.tile([1, 512], F32R, tag="mu", name="mu_t")
        var_t = rowp.tile([1, 512], F32R, tag="var", name="var_t")
        rstd_t = rowp.tile([1, 512], F32R, tag="rstd", name="rstd_t")
        mu, var, rstd = mu_t[0:1, :], var_t[0:1, :], rstd_t[0:1, :]
        nc.vector.tensor_scalar_mul(mu, psum_s[:], 1.0 / D)
        # D*var = sum(x^2) - sum(x)*mu ; fold 1/D into the sqrt scale
        nc.vector.tensor_tensor(out=var, in0=psum_s[:], in1=mu, op=ALU.mult)
        nc.vector.tensor_tensor(out=var, in0=psum_q[:], in1=var,
                                op=ALU.subtract)
        nc.scalar.activation(var, var, AF.Sqrt, scale=1.0 / D,
                             bias=eps_ln[0:1, 0:1])
        nc.vector.reciprocal(rstd, var)
        pmu_ps = ps_sc.tile([P, 512], F32, tag="sc", name="pmu")
        nc.tensor.matmul(pmu_ps[:], ones_row_r[:], mu, start=True, stop=True)
        prstd_ps = ps_sc.tile([P, 512], F32, tag="sc", name="prstd")
        nc.tensor.matmul(prstd_ps[:], ones_row_r[:], rstd, start=True,
                         stop=True)
        pmu = scratch.tile([P, 512], F32, tag="bcast_sb", name="pmu_sb")
        nc.vector.tensor_copy(pmu[:], pmu_ps[:])
        prstd = scratch.tile([P, 512], F32, tag="bcast_sb", name="prstd_sb")
        nc.vector.tensor_copy(prstd[:], prstd_ps[:])
        qsl = slice(qb * 512, (qb + 1) * 512)
        for m in range(DK):
            t = scratch.tile([P, 512], F32, tag="lnt")
            nc.vector.tensor_tensor(out=t[:], in0=src_t[:, m, qsl],
                                    in1=pmu[:], op=ALU.subtract)
            nc.vector.tensor_tensor(out=t[:], in0=t[:], in1=prstd[:],
                                    op=ALU.mult)
            nc.vector.tensor_scalar(
                out=xn1T(m)[:, qsl], in0=t[:],
                scalar1=vecs[:, C_G1 + m:C_G1 + m + 1],
                scalar2=vecs[:, C_BE1 + m:C_BE1 + m + 1],
                op0=ALU.mult, op1=ALU.add,
            )
            nc.vector.tensor_copy(xn1_bf(m)[:, qsl], xn1T(m)[:, qsl])

    for qb in range(QB):
        ln1_block(qb)
    if _nphase < 7:
        return

    # ---- FFN (per q-half to halve y1 SBUF) ------------------------------
    x2T = pA.tile([P, DK, M], F32R, tag="chA", name="x2T")
    for qb in range(QB):
        y1 = pV.tile([P, DFF // P, 512], BF16, tag="chV", name=f"y1_{qb}")
        for mfg in range(8):  # groups of 4 dff tiles
            psums = [ps_acc.tile([P, 512], F32, tag="acc", name=f"pf{mfg}_{i}")
                     for i in range(4)]
            for dk in range(DK):
                w1s = wff.tile([P, 512], BF16, tag="w1s", name="w1s")
                nc.sync.dma_start(
                    w1s[:], w1_t[dk][:, mfg * 512:(mfg + 1) * 512]
                )
                for mf in range(4):
                    nc.tensor.matmul(
                        psums[mf][:],
                        w1s[:, mf * 128:(mf + 1) * 128],
                        xn1_bf(dk)[:, qb * 512:(qb + 1) * 512],
                        start=(dk == 0),
                        stop=(dk == DK - 1),
                    )
            for mf in range(4):
                mfg_g = mfg * 4 + mf
                nc.scalar.activation(
                    y1[:, mfg_g, :], psums[mf][:], AF.Relu,
                    bias=vecs[:, C_B1 + mfg_g:C_B1 + mfg_g + 1],
                )
        for mg in range(2):  # groups of 4 d_model tiles
            psums = [ps_acc.tile([P, 512], F32, tag="acc", name=f"pg{mg}_{i}")
                     for i in range(4)]
            for dk in range(DFF // P):
                w2s = wff.tile([P, 512], BF16, tag="w2s", name="w2s")
                nc.sync.dma_start(w2s[:], w2_t[dk][:, mg * 512:(mg + 1) * 512])
                for m2 in range(4):
                    nc.tensor.matmul(
                        psums[m2][:],
                        w2s[:, m2 * 128:(m2 + 1) * 128],
                        y1[:, dk, :],
                        start=(dk == 0),
                        stop=(dk == DFF // P - 1),
                    )
            for m2 in range(4):
                m = mg * 4 + m2
                dst = x2T[:, m, qb * 512:(qb + 1) * 512]
                nc.vector.tensor_scalar_add(
                    dst, psums[m2][:], vecs[:, C_B2 + m:C_B2 + m + 1]
                )
                nc.vector.tensor_tensor(
                    out=dst, in0=dst, in1=xn1T(m)[:, qb * 512:(qb + 1) * 512],
                    op=ALU.add,
                )

    if _nphase < 8:
        return
    # ---- transpose to natural layout, LayerNorm2, output ----------------
    gb_bcast = pB.tile([P, 2, D], F32, tag="chB", name="gb")
    for v, row in enumerate((g2_row, be2_row)):
        for half in range(2):
            pb = ps_sc.tile([P, 512], F32, tag="sc", name=f"gb{v}_{half}")
            nc.tensor.matmul(
                pb[:], ones_row_r[:],
                row[:, half * 512:(half + 1) * 512],
                start=True, stop=True,
            )
            nc.vector.tensor_copy(gb_bcast[:, v, half * 512:(half + 1) * 512],
                                  pb[:])

    for qt in range(M // P):
        natt = scratch.tile([P, D], F32, tag="nat", name=f"nat{qt}")
        nat = natt[:, :]
        for m in range(DK):
            ptr = ps_sc.tile([P, P], F32, tag="sc", name=f"ptr{m}")
            nc.tensor.transpose(
                _r(ptr[:]), x2T[:, m, qt * 128:(qt + 1) * 128], identity_r[:]
            )
            nc.vector.tensor_copy(nat[:, m * 128:(m + 1) * 128], ptr[:])
        ssum = scr1.tile([P, 1], F32, tag="nsum")
        nc.vector.tensor_reduce(ssum[:], nat[:], axis=AX.X, op=ALU.add)
        ssq = scr1.tile([P, 1], F32, tag="nssq")
        for half in range(2):
            sqh = scratch.tile([P, 512], F32, tag="sq", name=f"nsq{half}")
            nath = nat[:, half * 512:(half + 1) * 512]
            nc.vector.tensor_tensor(out=sqh[:], in0=nath, in1=nath,
                                    op=ALU.mult)
            if half == 0:
                nc.vector.tensor_reduce(ssq[:], sqh[:], axis=AX.X, op=ALU.add)
            else:
                s2 = scr1.tile([P, 1], F32, tag="nssq2")
                nc.vector.tensor_reduce(s2[:], sqh[:], axis=AX.X, op=ALU.add)
                nc.vector.tensor_tensor(out=ssq[:], in0=ssq[:], in1=s2[:],
                                        op=ALU.add)
        mu = scr1.tile([P, 1], F32, tag="nmu")
        nc.vector.tensor_scalar_mul(mu[:], ssum[:], 1.0 / D)
        msq = scr1.tile([P, 1], F32, tag="nmsq")
        nc.vector.tensor_scalar_mul(msq[:], ssq[:], 1.0 / D)
        var = scr1.tile([P, 1], F32, tag="nvar")
        nc.vector.tensor_tensor(out=var[:], in0=mu[:], in1=mu[:], op=ALU.mult)
        nc.vector.tensor_tensor(out=var[:], in0=msq[:], in1=var[:],
                                op=ALU.subtract)
        std = scr1.tile([P, 1], F32, tag="nstd")
        nc.scalar.activation(std[:], var[:], AF.Sqrt, bias=eps_ln[:, 0:1])
        rstd = scr1.tile([P, 1], F32, tag="nrstd")
        nc.vector.reciprocal(rstd[:], std[:])
        nc.vector.tensor_scalar(
            out=nat[:], in0=nat[:], scalar1=mu[:], scalar2=rstd[:],
            op0=ALU.subtract, op1=ALU.mult,
        )
        nc.vector.tensor_tensor(out=nat[:], in0=nat[:], in1=gb_bcast[:, 0, :],
                                op=ALU.mult)
        nc.vector.tensor_tensor(out=nat[:], in0=nat[:], in1=gb_bcast[:, 1, :],
                                op=ALU.add)
        nc.sync.dma_start(out_d[qt * 128:(qt + 1) * 128, :], nat)

_NC_CACHE = None


def build_nc():
    global _NC_CACHE
    if _NC_CACHE is not None:
        return _NC_CACHE
    from contextlib import ExitStack

    nc = bacc.Bacc("TRN2", target_bir_lowering=False, debug=False,
                   num_devices=NCORES)
    with tile.TileContext(nc) as tc:
        with ExitStack() as ctx:
            _build_body(ctx, tc)
    nc.compile()
    _NC_CACHE = nc
    return nc


def _pack_vec(v, ntiles):
    return np.ascontiguousarray(np.asarray(v, np.float32).reshape(ntiles, P).T)


def make_in_maps(inputs):
    f32 = lambda a: np.ascontiguousarray(np.asarray(a, np.float32))
    bf = lambda a: np.ascontiguousarray(
        np.asarray(a, np.float32).astype(ml_dtypes.bfloat16)
    )
    x = f32(inputs["x"])
    xT = np.ascontiguousarray(x.transpose(0, 2, 1))  # (B, D, S)

    vecs = np.zeros((P, VECS_COLS), np.float32)
    vecs[:, C_BK:C_BK + 8] = _pack_vec(inputs["bk"], 8)
    vecs[:, C_BQ:C_BQ + 8] = _pack_vec(inputs["bq"], 8)
    vecs[:, C_BV:C_BV + 8] = _pack_vec(inputs["bv"], 8)
    vecs[:, C_BO:C_BO + 8] = _pack_vec(inputs["bo"], 8)
    vecs[:, C_B2:C_B2 + 8] = _pack_vec(inputs["b2"], 8)
    vecs[:, C_G1:C_G1 + 8] = _pack_vec(inputs["g1"], 8)
    vecs[:, C_BE1:C_BE1 + 8] = _pack_vec(inputs["be1"], 8)
    vecs[:, C_B1:C_B1 + 32] = _pack_vec(inputs["b1"], 32)
    bv_t = _pack_vec(inputs["bv"], 8)  # (128, 8)
    vecs[0:64, C_BVL:C_BVL + 8] = bv_t[64:128, :]

    shared = {
        "wqT": f32(np.asarray(inputs["wq"], np.float32).T),
        "wkT": f32(np.asarray(inputs["wk"], np.float32).T),
        "wvT": bf(np.asarray(inputs["wv"], np.float32).T),
        "woT": bf(np.asarray(inputs["wo"], np.float32).T),
        "w1T": bf(np.asarray(inputs["w1"], np.float32).T),
        "w2T": bf(np.asarray(inputs["w2"], np.float32).T),
        "vecs": vecs,
        "g2b2": np.ascontiguousarray(
            np.stack([f32(inputs["g2"]), f32(inputs["be2"])])
        ),
    }
    in_maps = []
    for c in range(NCORES):
        b, half = c // 2, c % 2
        m = dict(shared)
        m["x_kvT"] = np.ascontiguousarray(xT[b])
        m["x_qT"] = np.ascontiguousarray(xT[b][:, half * M:(half + 1) * M])
        in_maps.append(m)
    return in_maps


def run(inputs, trace=False, **kw):
    from concourse.bass_utils import run_bass_kernel_spmd

    nc = build_nc()
    in_maps = make_in_maps(inputs)
    res = run_bass_kernel_spmd(nc, in_maps, core_ids=list(range(NCORES)),
                               trace=trace, **kw)
    out = np.empty((B, S, D), np.float32)
    for c in range(NCORES):
        b, half = c // 2, c % 2
        out[b, half * M:(half + 1) * M, :] = res.results[c]["out"]
    return out, res


def kernel(**inputs):
    out, _ = run(inputs)
    return out
